# revision 1
# baseline (speedup 1.0000x reference)
"""Trainium2 Bass kernel for nn_Attention2D: 2D attention over spatial axis.

Reference computation (per batch element b):
  qkv = w_qkv @ x          (1x1 conv == channel GEMM), x: [256, 4096]
  q,k,v: [8 heads, 64, 4096];  q *= 64**-0.5
  sim[h,i,j] = sum_n q[h,i,n] k[h,j,n]   (contraction over SPATIAL n=4096)
  attn = softmax(sim, axis=j)
  out[h,i,n] = sum_j attn[h,i,j] v[h,j,n]
  y = w_out @ out + b_out

Sharding: data-parallel over batch, 16 elems / 8 cores = 2 per core.

Per-core dataflow (fp16 matmuls, fp32 PSUM accumulation):
  - qT,kT produced TRANSPOSED [n, 512] via x-stationary GEMM (sim needs
    n on partitions); q-scale folded into W_q on host.
  - v produced natural [512, n] via W-stationary GEMM.
  - sim per head-pair: col-packed MMs (head A -> psum[0:64], head B ->
    psum[64:128]) accumulated over 32 n-chunks.
  - softmax over free dim j; unnormalized exp written block-diagonally,
    one DMA-transpose gives attnT; 1/Z applied after attn@v.
  - attn@v: row+col diagonal packed MMs vs natural v tiles.
  - projection: W_outT-stationary GEMM + per-partition bias add.
"""
import numpy as np

HEADS = 8
DH = 64
DIM = 256
HIDDEN = 512
B = 16
N = 4096            # h*w = 64*64
N_CORES = 8
B_PER_CORE = B // N_CORES
NT = N // 512       # 8 moving tiles of 512
NCH = N // 128      # 32 contraction chunks of 128
PAIRS = HEADS // 2  # 4 head pairs
CC = DIM // 128     # 2 channel chunks
KC = HIDDEN // 128  # 4 hidden chunks

_nc_cache = {}


def _build():
    if "nc" in _nc_cache:
        return _nc_cache["nc"]
    from contextlib import ExitStack
    import concourse.bacc as bacc
    import concourse.tile as tile
    from concourse import mybir

    f16 = mybir.dt.float16
    f32 = mybir.dt.float32

    nc = bacc.Bacc("TRN2", target_bir_lowering=False, debug=False,
                   num_devices=N_CORES)
    x_d = nc.dram_tensor("x", [B_PER_CORE, DIM, N], f16, kind="ExternalInput").ap()
    wqk_d = nc.dram_tensor("wqk", [DIM, 2 * HIDDEN], f16, kind="ExternalInput").ap()
    wv_d = nc.dram_tensor("wv", [DIM, HIDDEN], f16, kind="ExternalInput").ap()
    wout_d = nc.dram_tensor("wout", [HIDDEN, DIM], f16, kind="ExternalInput").ap()
    b_d = nc.dram_tensor("b", [DIM], f32, kind="ExternalInput").ap()
    y_d = nc.dram_tensor("y", [B_PER_CORE, DIM, N], f32, kind="ExternalOutput").ap()

    with tile.TileContext(nc) as tc, ExitStack() as ctx:
        consts = ctx.enter_context(tc.tile_pool(name="consts", bufs=1))
        xp = ctx.enter_context(tc.tile_pool(name="xp", bufs=2))
        qkp = ctx.enter_context(tc.tile_pool(name="qkp", bufs=1))
        vp = ctx.enter_context(tc.tile_pool(name="vp", bufs=1))
        outp = ctx.enter_context(tc.tile_pool(name="outp", bufs=1))
        smallp = ctx.enter_context(tc.tile_pool(name="smallp", bufs=4))
        stagep = ctx.enter_context(tc.tile_pool(name="stagep", bufs=4))
        psqk = ctx.enter_context(tc.tile_pool(name="psqk", bufs=3, space="PSUM"))
        pss = ctx.enter_context(tc.tile_pool(name="pss", bufs=2, space="PSUM"))
        pbig = ctx.enter_context(tc.tile_pool(name="pbig", bufs=3, space="PSUM"))

        # ---- weights (loaded once) ----
        wqk_t = consts.tile([128, CC, 2 * HIDDEN], f16)
        nc.sync.dma_start(out=wqk_t[:], in_=wqk_d.rearrange("(c p) o -> p c o", p=128))
        wv_t = consts.tile([128, CC, HIDDEN], f16)
        nc.sync.dma_start(out=wv_t[:], in_=wv_d.rearrange("(c p) o -> p c o", p=128))
        wout_t = consts.tile([128, KC, DIM], f16)
        nc.sync.dma_start(out=wout_t[:], in_=wout_d.rearrange("(k p) o -> p k o", p=128))
        b_t = consts.tile([128, 2], f32)
        nc.sync.dma_start(out=b_t[:], in_=b_d.rearrange("(m p) -> p m", p=128))

        Exp = mybir.ActivationFunctionType.Exp
        X = mybir.AxisListType.X

        for e in range(B_PER_CORE):
            # ---- load x (split into 4 column pieces for earlier start) ----
            x_t = xp.tile([128, CC, N], f16, tag="x")
            x_src = x_d[e].rearrange("(c p) n -> p c n", p=128)
            for g in range(4):
                nc.sync.dma_start(out=x_t[:, :, g * 1024:(g + 1) * 1024],
                                  in_=x_src[:, :, g * 1024:(g + 1) * 1024])

            # ---- qT / kT GEMM (x chunks stationary) ----
            qT_t = qkp.tile([128, NCH, HIDDEN], f16, tag="qT")
            kT_t = qkp.tile([128, NCH, HIDDEN], f16, tag="kT")
            for t in range(NCH):
                ps_q = psqk.tile([128, HIDDEN], f32, tag="psqk")
                ps_k = psqk.tile([128, HIDDEN], f32, tag="psqk")
                for c in range(CC):
                    lhsT = x_t[:, c, t * 128:(t + 1) * 128]
                    nc.tensor.matmul(ps_q[:], lhsT, wqk_t[:, c, 0:HIDDEN],
                                     start=(c == 0), stop=(c == CC - 1))
                    nc.tensor.matmul(ps_k[:], lhsT, wqk_t[:, c, HIDDEN:2 * HIDDEN],
                                     start=(c == 0), stop=(c == CC - 1))
                nc.vector.tensor_copy(qT_t[:, t, :], ps_q[:])
                nc.vector.tensor_copy(kT_t[:, t, :], ps_k[:])

            # ---- v GEMM (WvT stationary) ----
            v_t = vp.tile([128, PAIRS, N], f16, tag="v")
            for p in range(PAIRS):
                for t8 in range(NT):
                    ps_v = pbig.tile([128, 512], f32, tag="pbig")
                    for c in range(CC):
                        nc.tensor.matmul(
                            ps_v[:], wv_t[:, c, p * 128:(p + 1) * 128],
                            x_t[:, c, t8 * 512:(t8 + 1) * 512],
                            start=(c == 0), stop=(c == CC - 1))
                    nc.vector.tensor_copy(v_t[:, p, t8 * 512:(t8 + 1) * 512], ps_v[:])

            # ---- per pair: sim + softmax + transpose ----
            attnTs = []
            rzs = []
            for p in range(PAIRS):
                ps_s = pss.tile([128, DH], f32, tag="pss")
                co = p * 128
                for t in range(NCH):
                    st, sp = t == 0, t == NCH - 1
                    nc.tensor.matmul(ps_s[0:64, :], qT_t[:, t, co:co + 64],
                                     kT_t[:, t, co:co + 64], start=st, stop=sp)
                    nc.tensor.matmul(ps_s[64:128, :], qT_t[:, t, co + 64:co + 128],
                                     kT_t[:, t, co + 64:co + 128], start=st, stop=sp)
                negmax = smallp.tile([128, 1], f32, tag="negmax")
                nc.vector.reduce_max(negmax[:], ps_s[:], axis=X, negate=True)
                esum = smallp.tile([128, 1], f32, tag="esum")
                attn_pad = smallp.tile([128, 128], f16, tag="attn_pad")
                nc.scalar.activation(attn_pad[0:64, 0:64], ps_s[0:64, :], Exp,
                                     bias=negmax[0:64, :], accum_out=esum[0:64, :])
                nc.scalar.activation(attn_pad[64:128, 64:128], ps_s[64:128, :], Exp,
                                     bias=negmax[64:128, :], accum_out=esum[64:128, :])
                rz = smallp.tile([128, 1], f32, tag="rz")
                nc.vector.reciprocal(rz[:], esum[:])
                attnT = smallp.tile([128, 128], f16, tag="attnT")
                nc.sync.dma_start_transpose(out=attnT[:], in_=attn_pad[:])
                attnTs.append(attnT)
                rzs.append(rz)

            # ---- attn@v (row+col diagonal packing) + 1/Z scale ----
            out_t = outp.tile([128, KC, N], f16, tag="out")
            for p in range(PAIRS):
                attnT, rz = attnTs[p], rzs[p]
                for t8 in range(NT):
                    ps_o = pbig.tile([128, 512], f32, tag="pbig")
                    nc.tensor.matmul(ps_o[0:64, :], attnT[0:64, 0:64],
                                     v_t[0:64, p, t8 * 512:(t8 + 1) * 512],
                                     start=True, stop=True)
                    nc.tensor.matmul(ps_o[64:128, :], attnT[64:128, 64:128],
                                     v_t[64:128, p, t8 * 512:(t8 + 1) * 512],
                                     start=True, stop=True)
                    nc.vector.tensor_scalar_mul(
                        out_t[:, p, t8 * 512:(t8 + 1) * 512], ps_o[:], rz[:])

            # ---- projection + bias ----
            for m in range(2):
                for t8 in range(NT):
                    ps_y = pbig.tile([128, 512], f32, tag="pbig")
                    for k in range(KC):
                        nc.tensor.matmul(
                            ps_y[:], wout_t[:, k, m * 128:(m + 1) * 128],
                            out_t[:, k, t8 * 512:(t8 + 1) * 512],
                            start=(k == 0), stop=(k == KC - 1))
                    y_stage = stagep.tile([128, 512], f32, tag="y_stage")
                    nc.vector.tensor_scalar_add(y_stage[:], ps_y[:], b_t[:, m:m + 1])
                    nc.sync.dma_start(
                        out=y_d[e, m * 128:(m + 1) * 128, t8 * 512:(t8 + 1) * 512],
                        in_=y_stage[:])

    nc.compile()
    _nc_cache["nc"] = nc
    return nc


def _prep_inputs(x, w_qkv, w_out, b_out):
    scale = DH ** (-0.5)
    wq = (w_qkv[0:HIDDEN] * scale).astype(np.float16)       # [512, 256]
    wk = w_qkv[HIDDEN:2 * HIDDEN].astype(np.float16)
    wv = w_qkv[2 * HIDDEN:3 * HIDDEN].astype(np.float16)
    wqk = np.concatenate([wq.T, wk.T], axis=1).copy()       # [256, 1024]
    wv_T = wv.T.copy()                                      # [256, 512]
    wout_T = w_out.T.astype(np.float16).copy()              # [512, 256]
    b = b_out.astype(np.float32)
    x16 = np.ascontiguousarray(x.reshape(B, DIM, N)).astype(np.float16)
    return x16, wqk, wv_T, wout_T, b


def _run(x, w_qkv, w_out, b_out, trace=False, tmpdir=None):
    from concourse.bass_utils import run_bass_kernel_spmd

    nc = _build()
    x16, wqk, wv_T, wout_T, b = _prep_inputs(x, w_qkv, w_out, b_out)
    in_maps = [
        {"x": x16[i * B_PER_CORE:(i + 1) * B_PER_CORE], "wqk": wqk, "wv": wv_T,
         "wout": wout_T, "b": b}
        for i in range(N_CORES)
    ]
    kw = {}
    if trace:
        kw = {"trace": True, "tmpdir": tmpdir}
    res = run_bass_kernel_spmd(nc, in_maps, core_ids=list(range(N_CORES)), **kw)
    y = np.concatenate([res.results[i]["y"] for i in range(N_CORES)], axis=0)
    return y.reshape(B, DIM, 64, 64), res


def kernel(x, w_qkv, w_out, b_out):
    y, _ = _run(np.asarray(x), np.asarray(w_qkv), np.asarray(w_out),
                np.asarray(b_out))
    return y


# revision 4
# speedup vs baseline: 1.1190x; 1.1190x over previous
"""Trainium2 Bass kernel for nn_Attention2D: 2D attention over spatial axis.

Reference computation (per batch element b):
  qkv = w_qkv @ x          (1x1 conv == channel GEMM), x: [256, 4096]
  q,k,v: [8 heads, 64, 4096];  q *= 64**-0.5
  sim[h,i,j] = sum_n q[h,i,n] k[h,j,n]   (contraction over SPATIAL n=4096)
  attn = softmax(sim, axis=j)
  out[h,i,n] = sum_j attn[h,i,j] v[h,j,n]
  y = w_out @ out + b_out

Sharding: data-parallel over batch, 16 elems / 8 cores = 2 per core.

Per-core dataflow (fp16 matmuls, fp32 PSUM accumulation):
  - q,k produced TRANSPOSED [n, 512] via x-stationary GEMM (sim needs n
    on partitions); q-scale folded into W_q on host. q and k GEMMs share
    one double-bank PSUM tile -> single wide psum->sbuf cast per n-tile.
  - v produced natural [512, n] via W-stationary GEMM.
  - sim per head-pair: ONE N=128 MM per chunk (both heads; off-diagonal
    blocks of the pair Gram matrix are computed-and-ignored, same cycles).
  - softmax over free dim j; unnormalized exp written block-diagonally,
    one DMA-transpose gives attnT; 1/Z folded into the attn@v psum->sbuf
    copy (per-partition scale).
  - attn@v: row+col diagonal packed MMs vs natural v tiles.
  - projection: W_outT-stationary GEMM, per-partition bias fused into the
    psum->sbuf copy.
  - post-PSUM traffic alternates DVE / ACT to balance both engines.
"""
import numpy as np

HEADS = 8
DH = 64
DIM = 256
HIDDEN = 512
B = 16
N = 4096            # h*w = 64*64
N_CORES = 8
B_PER_CORE = B // N_CORES
NT = N // 512       # 8 moving tiles of 512
NTP = NT // 2       # 4 double-tiles of 1024
NCH = N // 128      # 32 contraction chunks of 128
PAIRS = HEADS // 2  # 4 head pairs
CC = DIM // 128     # 2 channel chunks
KC = HIDDEN // 128  # 4 hidden chunks

_nc_cache = {}


def _build():
    if "nc" in _nc_cache:
        return _nc_cache["nc"]
    from contextlib import ExitStack
    import concourse.bacc as bacc
    import concourse.tile as tile
    from concourse import mybir

    f16 = mybir.dt.float16
    f32 = mybir.dt.float32
    Copy = mybir.ActivationFunctionType.Copy
    Exp = mybir.ActivationFunctionType.Exp
    X = mybir.AxisListType.X

    nc = bacc.Bacc("TRN2", target_bir_lowering=False, debug=False,
                   num_devices=N_CORES)
    x_d = nc.dram_tensor("x", [B_PER_CORE, DIM, N], f16, kind="ExternalInput").ap()
    wqk_d = nc.dram_tensor("wqk", [DIM, 2 * HIDDEN], f16, kind="ExternalInput").ap()
    wv_d = nc.dram_tensor("wv", [DIM, HIDDEN], f16, kind="ExternalInput").ap()
    wout_d = nc.dram_tensor("wout", [HIDDEN, DIM], f16, kind="ExternalInput").ap()
    b_d = nc.dram_tensor("b", [DIM], f32, kind="ExternalInput").ap()
    y_d = nc.dram_tensor("y", [B_PER_CORE, DIM, N], f32, kind="ExternalOutput").ap()

    with tile.TileContext(nc) as tc, ExitStack() as ctx:
        consts = ctx.enter_context(tc.tile_pool(name="consts", bufs=1))
        xp = ctx.enter_context(tc.tile_pool(name="xp", bufs=2))
        qkp = ctx.enter_context(tc.tile_pool(name="qkp", bufs=1))
        vp = ctx.enter_context(tc.tile_pool(name="vp", bufs=1))
        outp = ctx.enter_context(tc.tile_pool(name="outp", bufs=1))
        smallp = ctx.enter_context(tc.tile_pool(name="smallp", bufs=4))
        stagep = ctx.enter_context(tc.tile_pool(name="stagep", bufs=2))
        pb = ctx.enter_context(tc.tile_pool(name="pb", bufs=3, space="PSUM"))
        pss = ctx.enter_context(tc.tile_pool(name="pss", bufs=2, space="PSUM"))

        # ---- weights (loaded once) ----
        wqk_t = consts.tile([128, CC, 2 * HIDDEN], f16)
        nc.sync.dma_start(out=wqk_t[:], in_=wqk_d.rearrange("(c p) o -> p c o", p=128))
        wv_t = consts.tile([128, CC, HIDDEN], f16)
        nc.sync.dma_start(out=wv_t[:], in_=wv_d.rearrange("(c p) o -> p c o", p=128))
        wout_t = consts.tile([128, KC, DIM], f16)
        nc.sync.dma_start(out=wout_t[:], in_=wout_d.rearrange("(k p) o -> p k o", p=128))
        b_t = consts.tile([128, 2], f32)
        nc.sync.dma_start(out=b_t[:], in_=b_d.rearrange("(m p) -> p m", p=128))

        def copy_cast(i, dst, src):
            # alternate big psum->sbuf casts between DVE and ACT
            if i % 2 == 0:
                nc.vector.tensor_copy(dst, src)
            else:
                nc.scalar.activation(dst, src, Copy)

        for e in range(B_PER_CORE):
            # ---- load x (8 column pieces for early start) ----
            x_t = xp.tile([128, CC, N], f16, tag="x")
            x_src = x_d[e].rearrange("(c p) n -> p c n", p=128)
            for g in range(8):
                nc.sync.dma_start(out=x_t[:, :, g * 512:(g + 1) * 512],
                                  in_=x_src[:, :, g * 512:(g + 1) * 512])

            # ---- q/k GEMM (x chunks stationary), fused double-bank psum ----
            qk_t = qkp.tile([128, NCH, 2 * HIDDEN], f16, tag="qk")
            for t in range(NCH):
                ps_qk = pb.tile([128, 1024], f32, tag="pb")
                for c in range(CC):
                    lhsT = x_t[:, c, t * 128:(t + 1) * 128]
                    nc.tensor.matmul(ps_qk[:, 0:512], lhsT, wqk_t[:, c, 0:HIDDEN],
                                     start=(c == 0), stop=(c == CC - 1))
                    nc.tensor.matmul(ps_qk[:, 512:1024], lhsT,
                                     wqk_t[:, c, HIDDEN:2 * HIDDEN],
                                     start=(c == 0), stop=(c == CC - 1))
                copy_cast(t, qk_t[:, t, :], ps_qk[:])

            # ---- per pair: sim + softmax + transpose ----
            attnTs = []
            rzs = []
            for p in range(PAIRS):
                ps_s = pss.tile([128, 128], f32, tag="pss")
                co = p * 128
                for t in range(NCH):
                    nc.tensor.matmul(ps_s[:], qk_t[:, t, co:co + 128],
                                     qk_t[:, t, 512 + co:512 + co + 128],
                                     start=(t == 0), stop=(t == NCH - 1))
                negmax = smallp.tile([128, 1], f32, tag="negmax")
                nc.vector.reduce_max(negmax[0:64, :], ps_s[0:64, 0:64],
                                     axis=X, negate=True)
                nc.vector.reduce_max(negmax[64:128, :], ps_s[64:128, 64:128],
                                     axis=X, negate=True)
                esum = smallp.tile([128, 1], f32, tag="esum")
                attn_pad = smallp.tile([128, 128], f16, tag="attn_pad")
                nc.scalar.activation(attn_pad[0:64, 0:64], ps_s[0:64, 0:64], Exp,
                                     bias=negmax[0:64, :], accum_out=esum[0:64, :])
                nc.scalar.activation(attn_pad[64:128, 64:128], ps_s[64:128, 64:128],
                                     Exp, bias=negmax[64:128, :],
                                     accum_out=esum[64:128, :])
                rz = smallp.tile([128, 1], f32, tag="rz")
                nc.vector.reciprocal(rz[:], esum[:])
                attnT = smallp.tile([128, 128], f16, tag="attnT")
                nc.sync.dma_start_transpose(out=attnT[:], in_=attn_pad[:])
                attnTs.append(attnT)
                rzs.append(rz)

            # ---- v GEMM (WvT stationary) ----
            v_t = vp.tile([128, PAIRS, N], f16, tag="v")
            for tp in range(NTP):
                for p in range(PAIRS):
                    ps_v = pb.tile([128, 1024], f32, tag="pb")
                    for c in range(CC):
                        w = wv_t[:, c, p * 128:(p + 1) * 128]
                        nc.tensor.matmul(ps_v[:, 0:512], w,
                                         x_t[:, c, tp * 1024:tp * 1024 + 512],
                                         start=(c == 0), stop=(c == CC - 1))
                        nc.tensor.matmul(ps_v[:, 512:1024], w,
                                         x_t[:, c, tp * 1024 + 512:(tp + 1) * 1024],
                                         start=(c == 0), stop=(c == CC - 1))
                    copy_cast(tp * PAIRS + p, v_t[:, p, tp * 1024:(tp + 1) * 1024],
                              ps_v[:])

            # ---- attn@v (row+col diagonal packing) + proj, per double-tile ----
            out_t = outp.tile([128, KC, N], f16, tag="out")
            for tp in range(NTP):
                for p in range(PAIRS):
                    attnT, rz = attnTs[p], rzs[p]
                    ps_o = pb.tile([128, 1024], f32, tag="pb")
                    for half in range(2):
                        sl = slice(tp * 1024 + half * 512, tp * 1024 + half * 512 + 512)
                        od = slice(half * 512, half * 512 + 512)
                        nc.tensor.matmul(ps_o[0:64, od], attnT[0:64, 0:64],
                                         v_t[0:64, p, sl], start=True, stop=True)
                        nc.tensor.matmul(ps_o[64:128, od], attnT[64:128, 64:128],
                                         v_t[64:128, p, sl], start=True, stop=True)
                    # 1/Z scale fused into the psum->sbuf copy
                    dst = out_t[:, p, tp * 1024:(tp + 1) * 1024]
                    if p % 2 == 0:
                        nc.vector.tensor_scalar_mul(dst, ps_o[:], rz[:])
                    else:
                        nc.scalar.mul(dst, ps_o[:], rz[:])

                for m in range(2):
                    ps_y = pb.tile([128, 1024], f32, tag="pb")
                    for k in range(KC):
                        w = wout_t[:, k, m * 128:(m + 1) * 128]
                        nc.tensor.matmul(ps_y[:, 0:512], w,
                                         out_t[:, k, tp * 1024:tp * 1024 + 512],
                                         start=(k == 0), stop=(k == KC - 1))
                        nc.tensor.matmul(ps_y[:, 512:1024], w,
                                         out_t[:, k, tp * 1024 + 512:(tp + 1) * 1024],
                                         start=(k == 0), stop=(k == KC - 1))
                    y_stage = stagep.tile([128, 1024], f32, tag="y_stage")
                    if m % 2 == 0:
                        nc.vector.tensor_scalar_add(y_stage[:], ps_y[:], b_t[:, m:m + 1])
                    else:
                        nc.scalar.add(y_stage[:], ps_y[:], b_t[:, m:m + 1])
                    nc.sync.dma_start(
                        out=y_d[e, m * 128:(m + 1) * 128, tp * 1024:(tp + 1) * 1024],
                        in_=y_stage[:])

    nc.compile()
    _nc_cache["nc"] = nc
    return nc


def _prep_inputs(x, w_qkv, w_out, b_out):
    scale = DH ** (-0.5)
    wq = (w_qkv[0:HIDDEN] * scale).astype(np.float16)       # [512, 256]
    wk = w_qkv[HIDDEN:2 * HIDDEN].astype(np.float16)
    wv = w_qkv[2 * HIDDEN:3 * HIDDEN].astype(np.float16)
    wqk = np.concatenate([wq.T, wk.T], axis=1).copy()       # [256, 1024]
    wv_T = wv.T.copy()                                      # [256, 512]
    wout_T = w_out.T.astype(np.float16).copy()              # [512, 256]
    b = b_out.astype(np.float32)
    x16 = np.ascontiguousarray(x.reshape(B, DIM, N)).astype(np.float16)
    return x16, wqk, wv_T, wout_T, b


def _run(x, w_qkv, w_out, b_out, trace=False, tmpdir=None):
    from concourse.bass_utils import run_bass_kernel_spmd

    nc = _build()
    x16, wqk, wv_T, wout_T, b = _prep_inputs(x, w_qkv, w_out, b_out)
    in_maps = [
        {"x": x16[i * B_PER_CORE:(i + 1) * B_PER_CORE], "wqk": wqk, "wv": wv_T,
         "wout": wout_T, "b": b}
        for i in range(N_CORES)
    ]
    kw = {}
    if trace:
        kw = {"trace": True, "tmpdir": tmpdir}
    res = run_bass_kernel_spmd(nc, in_maps, core_ids=list(range(N_CORES)), **kw)
    y = np.concatenate([res.results[i]["y"] for i in range(N_CORES)], axis=0)
    return y.reshape(B, DIM, 64, 64), res


def kernel(x, w_qkv, w_out, b_out):
    y, _ = _run(np.asarray(x), np.asarray(w_qkv), np.asarray(w_out),
                np.asarray(b_out))
    return y
